# revision 24
# baseline (speedup 1.0000x reference)
"""CMHSA (1x1-conv multi-head self-attention with a head-mixing 1x1 conv and
instance-norm on the attention maps) as a Trainium2 Bass kernel on 8
NeuronCores.

Reference math (B=4, C=512, T=1024, HEADS=8, hd=64):
  xf = x[b] as [C, T];  q/k/v = W @ xf;  per head h: S_h = q_h^T k_h * hd^-.5
  S'_g = sum_h w_head[g,h] S_h            (head-mixing 1x1 conv)
  A = softmax(S'_g, axis=-1)
  A = instnorm(A) * gamma_g + beta_g      (stats over the whole [T,T] map)
  out = (A @ v_g^T).view(b, t, c) @ w_proj.T + b_proj -> [B, C, H, W]

Transformations:
  * Head-mixing folds into Q: S'_g = (alpha_g . q)^T k with per-channel
    scale alpha_g[o] = w_head[g, o//64].  Each (b, g) map becomes fully
    independent -> 32 maps over 8 cores, 4 maps/core, zero collectives.
  * Logits are ~N(0,1): softmax without max-subtraction is safe.
  * Attention is computed transposed (S^T[T, q]) so the T (softmax) axis is
    the PE contraction axis; softmax row-sums come out of the AV matmul by
    appending 64 ones-columns to the stationary [v_g | 1]: PSUM rows 0-63 =
    v @ E, rows 64-127 = rowsum (pre-broadcast).  A second column-tiled
    matmul with an all-ones stationary reduces E^2 for the variance.
  * gamma/inv_std/beta/b_proj and the constant (beta - a*mu) * sum_T v term
    fold into a host epilogue given per-map sum_q sqsum/rowsum^2, which the
    device emits as a tiny second output.
  * The G map is written parity-split (g2[64*(s%2)+d, 128*(s//2)+i] =
    G[d, 8i+s]) so the projection contracts 128 partitions per matmul:
    4 matmuls/map instead of 8 while still realizing torch's
    .view(b, t, c) shuffle for free.
  * The 64 ones-columns of the AV stationary give the rowsum already
    broadcast across PSUM partitions 64-127 for free (PE cost is
    moving-rows only), so no separate broadcast matmul is needed: the
    reciprocal chain runs directly on those rows.  The sq stationary is
    sliced to 1 column (LDW is self-loading per matmul; narrower loads
    faster), and the static ones-columns of V' fill once outside the
    rep loop.
"""

import numpy as np

import concourse.bass as bass
import concourse.tile as tile
import concourse.mybir as mybir
from concourse import bacc
from concourse.bass_utils import run_bass_kernel_spmd

F32 = mybir.dt.float32
F32R = mybir.dt.float32r

B, C, HH, WW = 4, 512, 32, 32
T = HH * WW          # 1024
HEADS, HD = 8, 64
EPS = 1e-5
SCALE = HD ** -0.5   # 1/8
NCORES = 8
GPC = HEADS // 2     # 4 maps (g values) per core; 2 cores per batch
CC = C // 128        # 4 contraction chunks
TB = T // 128        # 8 T-blocks
MU = 1.0 / T

_prog_cache = {}


def build_program(reps=1):
    """Build + compile the SPMD Bass program (one NEFF, same for all cores).

    reps>1 repeats the whole compute body (for wall-clock timing via
    wall(reps=R) - wall(reps=1)); input loads run once."""
    if reps in _prog_cache:
        return _prog_cache[reps]

    nc = bacc.Bacc("TRN2", target_bir_lowering=False, debug=False,
                   num_devices=NCORES)

    x_d = nc.dram_tensor("x", [C, T], F32R, kind="ExternalInput")
    wq_d = nc.dram_tensor("wqT", [C, C], F32R, kind="ExternalInput")
    wk_d = nc.dram_tensor("wkT", [C, C], F32R, kind="ExternalInput")
    wv_d = nc.dram_tensor("wvT", [C, GPC * HD], F32R, kind="ExternalInput")
    wp_d = nc.dram_tensor("wpT", [128, 4 * C], F32R, kind="ExternalInput")
    al_d = nc.dram_tensor("alphas", [128, CC * GPC], F32, kind="ExternalInput")
    on_d = nc.dram_tensor("ones", [128, 128], F32R, kind="ExternalInput")
    out_d = nc.dram_tensor("out", [GPC * 128, C], F32, kind="ExternalOutput")
    s2_d = nc.dram_tensor("s2", [GPC, 2], F32, kind="ExternalOutput")

    with tile.TileContext(nc) as tc:
        with (
            tc.tile_pool(name="persist", bufs=1) as persist,
            tc.tile_pool(name="qg", bufs=2) as qg_pool,
            tc.tile_pool(name="e", bufs=3) as e_pool,
            tc.tile_pool(name="e2", bufs=3) as e2_pool,
            tc.tile_pool(name="g", bufs=2) as g_pool,
            tc.tile_pool(name="st", bufs=2) as st_pool,
            tc.tile_pool(name="qkps", bufs=2, space="PSUM") as qk_ps,
            tc.tile_pool(name="avps", bufs=2, space="PSUM") as av_ps,
        ):
            # ---------------- load inputs ----------------
            x_sb = persist.tile([128, CC * T], F32R)   # x[c,t]; chunk cc at cols cc*T
            for cc in range(CC):
                for th in range(2):
                    nc.sync.dma_start(
                        x_sb[:, cc * T + th * 512:cc * T + (th + 1) * 512],
                        x_d[cc * 128:(cc + 1) * 128,
                            th * 512:(th + 1) * 512])
            wq_sb = persist.tile([128, CC * C], F32R)  # w_q.T/8; chunk cc at cols cc*C
            wk_sb = persist.tile([128, CC * C], F32R)
            for w_sb, w_d in ((wq_sb, wq_d), (wk_sb, wk_d)):
                for cc in range(CC):
                    nc.sync.dma_start(w_sb[:, cc * C:(cc + 1) * C],
                                      w_d[cc * 128:(cc + 1) * 128, :])
            wv_sb = persist.tile([128, CC * GPC * HD], F32R)  # this core's v heads
            for cc in range(CC):
                nc.sync.dma_start(
                    wv_sb[:, cc * GPC * HD:(cc + 1) * GPC * HD],
                    wv_d[cc * 128:(cc + 1) * 128, :])
            # wp2[64*(s%2)+d, 512*(s//2)+c] = w_proj[c, 64*s+d]; pairs the 8
            # stride-8 projection chunks into 4 matmuls of 128-deep contraction
            wp_sb = persist.tile([128, 4 * C], F32R)
            nc.sync.dma_start(wp_sb[:], wp_d[:])
            al_sb = persist.tile([128, CC * GPC], F32)
            nc.sync.dma_start(al_sb[:], al_d[:])
            ones_sb = persist.tile([128, 128], F32R)
            nc.sync.dma_start(ones_sb[:], on_d[:])

            # V' = [v-slices | ones] layout; the ones columns are static, so
            # fill them once outside the rep loop.
            # vp block tb (512 cols): [gi*128, gi*128+64) = V^T[:, gi*64..]
            #                         [gi*128+64, gi*128+128) = ones
            vp_sb = persist.tile([128, TB * 512], F32R)
            vp_v = vp_sb[:].rearrange("p (t g k) -> p t g k", g=GPC, k=128)
            for tb in range(TB):
                nc.sync.dma_start(
                    vp_v[:, tb, :, 64:128],
                    bass.AP(tensor=on_d, offset=0,
                            ap=[[128, 128], [0, GPC], [1, 64]]))

            for _rep in range(reps):
                # ---------------- Q, K = W @ x ----------------
                q_sb = persist.tile([128, CC * T], F32R)   # Q[o,t]; chunk ob at cols ob*T
                k_sb = persist.tile([128, CC * T], F32R)
                for w_sb, dst in ((wq_sb, q_sb), (wk_sb, k_sb)):
                    for ob in range(4):
                        ps = qk_ps.tile([128, 1024], F32, tag="mmps", name="qkv_ps")
                        for th in range(2):
                            for cc in range(CC):
                                nc.tensor.matmul(
                                    ps[:, th * 512:(th + 1) * 512],
                                    (w_sb[:, cc * C + ob * 128:
                                            cc * C + (ob + 1) * 128]),
                                    (x_sb[:, cc * T + th * 512:
                                            cc * T + th * 512 + 512]),
                                    start=(cc == 0), stop=(cc == CC - 1))
                        nc.scalar.copy(dst[:, ob * T:(ob + 1) * T], ps[:])

                # ---------------- V' v-slices per T-block ----------------
                for tb in range(TB):
                    ps = qk_ps.tile([128, 1024], F32, tag="mmps", name="vt_ps")
                    for cc in range(CC):
                        nc.tensor.matmul(
                            ps[:, 0:GPC * HD],
                            (x_sb[:, cc * T + tb * 128:cc * T + (tb + 1) * 128]),
                            (wv_sb[:, cc * GPC * HD:(cc + 1) * GPC * HD]),
                            start=(cc == 0), stop=(cc == CC - 1))
                    nc.vector.tensor_copy(
                        vp_v[:, tb, :, 0:64],
                        ps[:, 0:GPC * HD].rearrange("p (g k) -> p g k", k=64))

                # ---------------- per-map pipeline ----------------
                # A single emission FIFO keeps every non-QK chunk of work
                # (AV/sqsum matmuls, epilogue, projection) trailing ~2 steps
                # behind the QK stream, across map boundaries, so the PE's
                # static order always has QK matmuls to chew while ACT (exp)
                # and the rowsum-reciprocal chain catch up.

                from collections import deque
                todo = deque()

                def emit_qg(gi):
                    qg_sb = qg_pool.tile([128, CC * T], F32R, tag="qg",
                                         name=f"qg{gi}")
                    for cc in range(CC):
                        nc.vector.tensor_scalar_mul(
                            qg_sb[:, cc * T:(cc + 1) * T],
                            q_sb[:, cc * T:(cc + 1) * T],
                            al_sb[:, cc * GPC + gi:cc * GPC + gi + 1])
                    return qg_sb

                def emit_avsq(gi, tb, avs, sqs, e_t, e2_t):
                    # Full [v | ones x64] stationary: PSUM rows 64-127 come
                    # out as the rowsum already broadcast across 64
                    # partitions (PE cost is moving-rows only), which makes
                    # a separate broadcast matmul unnecessary.  The sq
                    # stationary stays 1 column (only row 0 is read).
                    for qh in range(2):
                        nc.tensor.matmul(
                            avs[qh][:, :],
                            vp_sb[:, tb * 512 + gi * 128:
                                  tb * 512 + (gi + 1) * 128],
                            e_t[:, qh * 512:(qh + 1) * 512],
                            start=(tb == 0), stop=(tb == TB - 1))
                        nc.tensor.matmul(
                            sqs[qh][0:1, :],
                            ones_sb[:, 0:1],
                            e2_t[:, qh * 512:(qh + 1) * 512],
                            start=(tb == 0), stop=(tb == TB - 1))

                def emit_epilogue(gi, avs, sqs, g2_sb):
                    # Copy av PSUM to SBUF immediately (releases the
                    # accumulator banks before the reciprocal chain).
                    s2_t = st_pool.tile([1, 2], F32, tag="s2_t", name="s2_t")
                    avc, rsum = [], []
                    for qh in range(2):
                        a_sb = st_pool.tile([128, 512], F32, tag="avc",
                                            name="a_sb", bufs=3)
                        nc.vector.tensor_copy(a_sb[0:64, :],
                                              avs[qh][0:64, :])
                        # the pre-broadcast rowsum rows, shifted to base
                        # partition 0 so later SBUF-SBUF tensor ops align
                        rs_sb = st_pool.tile([128, 512], F32, tag="rsum",
                                             name="rs_sb", bufs=3)
                        nc.vector.tensor_copy(rs_sb[0:64, :],
                                              avs[qh][64:128, :])
                        avc.append(a_sb)
                        rsum.append(rs_sb)
                    for qh in range(2):
                        # r = 1/rowsum via exp(-ln .) on ACT (the pinned
                        # table set has both), computed directly on the 64
                        # pre-broadcast rowsum partitions (ACT cost is
                        # free-size-based, so the width is free).
                        lnr = st_pool.tile([128, 512], F32, tag="lnr",
                                           name="lnr")
                        nc.scalar.activation(lnr[0:64, :], rsum[qh][0:64, :],
                                             mybir.ActivationFunctionType.Ln)
                        rrow = st_pool.tile([128, 512], F32, tag="rrow",
                                            name="rrow")
                        nc.scalar.activation(rrow[0:64, :], lnr[0:64, :],
                                             mybir.ActivationFunctionType.Exp,
                                             scale=-1.0)
                        # G = (v@E) * r, written parity-split so the
                        # projection can contract 128 partitions at a time:
                        # g2[64*(s%2)+d, 128*(s//2)+i] = G[d, q=8i+s]
                        av_r = avc[qh][:].rearrange("p (i j k) -> p k j i",
                                                    j=4, k=2)
                        rb_r = rrow[:].rearrange("p (i j k) -> p k j i",
                                                 j=4, k=2)
                        g2_r = g2_sb[:].rearrange("p (j i) -> p j i", i=128)
                        for par in range(2):
                            nc.vector.tensor_tensor(
                                g2_r[64 * par:64 * par + 64, :,
                                     qh * 64:qh * 64 + 64],
                                av_r[0:64, par, :, :],
                                rb_r[0:64, par, :, :],
                                mybir.AluOpType.mult)
                        # s2[qh] = sum_q (sum_T E^2) * r^2, reading sum_T E^2
                        # straight off PSUM row 0 (all sq rows identical).
                        r2 = st_pool.tile([128, 512], F32, tag="r2",
                                          name="r2")
                        nc.vector.tensor_mul(r2[0:1, :], rrow[0:1, :],
                                             rrow[0:1, :])
                        u = st_pool.tile([128, 512], F32, tag="u", name="u")
                        nc.vector.tensor_tensor(u[0:1, :], r2[0:1, :],
                                                sqs[qh][0:1, :],
                                                mybir.AluOpType.mult)
                        nc.vector.reduce_sum(s2_t[0:1, qh:qh + 1],
                                             u[0:1, :],
                                             axis=mybir.AxisListType.X)
                    nc.sync.dma_start(s2_d[gi:gi + 1, :], s2_t[0:1, :])

                def emit_proj(gi, g2_sb):
                    # out^T[i,c] = sum_s sum_d G[d, 8i+s] * wp[d, s*512+c],
                    # two s-parities stacked per 128-deep contraction chunk
                    p_ps = av_ps.tile([128, 512], F32, tag="av", name="p_ps")
                    for j2 in range(4):
                        nc.tensor.matmul(p_ps[:],
                                         g2_sb[:, j2 * 128:(j2 + 1) * 128],
                                         wp_sb[:, j2 * C:(j2 + 1) * C],
                                         start=(j2 == 0), stop=(j2 == 3))
                    stage = st_pool.tile([128, 512], F32, tag="stage",
                                         name="stage", bufs=2)
                    nc.scalar.copy(stage[:], p_ps[:])
                    nc.sync.dma_start(out_d[gi * 128:(gi + 1) * 128, :],
                                      stage[:])

                qg_next = emit_qg(0)
                for gi in range(GPC):
                    qg_sb = qg_next
                    avs = tuple(av_ps.tile([128, 512], F32, tag="av",
                                           name=f"av{qh}") for qh in range(2))
                    sqs = tuple(av_ps.tile([128, 512], F32, tag="sq",
                                           name=f"sq{qh}") for qh in range(2))

                    for tb in range(TB):
                        s_ps = qk_ps.tile([128, 1024], F32, tag="mmps",
                                          name="s_ps")
                        for qh in range(2):
                            for oc in range(CC):
                                nc.tensor.matmul(
                                    s_ps[:, qh * 512:(qh + 1) * 512],
                                    k_sb[:, oc * T + tb * 128:
                                         oc * T + (tb + 1) * 128],
                                    qg_sb[:, oc * T + qh * 512:
                                          oc * T + qh * 512 + 512],
                                    start=(oc == 0), stop=(oc == CC - 1))
                        e_t = e_pool.tile([128, 1024], F32R)
                        nc.scalar.activation(e_t[:], s_ps[:],
                                             mybir.ActivationFunctionType.Exp)
                        e2_t = e2_pool.tile([128, 1024], F32R)
                        if tb % 2 == 0:
                            nc.vector.tensor_mul(e2_t[:], e_t[:], e_t[:])
                        else:
                            nc.scalar.activation(
                                e2_t[:], s_ps[:],
                                mybir.ActivationFunctionType.Exp, scale=2.0)
                        todo.append(lambda gi=gi, tb=tb, a=avs, s=sqs,
                                    e=e_t, e2=e2_t:
                                    emit_avsq(gi, tb, a, s, e, e2))
                        if tb == 4 and gi + 1 < GPC:
                            qg_next = emit_qg(gi + 1)
                        while len(todo) > 2:
                            todo.popleft()()
                    g2_sb = g_pool.tile([128, 512], F32R)
                    todo.append(lambda gi=gi, a=avs, s=sqs, g=g2_sb:
                                emit_epilogue(gi, a, s, g))
                    todo.append(lambda gi=gi, g=g2_sb: emit_proj(gi, g))
                while todo:
                    todo.popleft()()

    _pin_act_table(nc)
    nc.compile()
    _prog_cache[reps] = nc
    return nc


def _pin_act_table(nc):
    """Make Exp/Ln/Copy resolvable only via natural_log_exp_and_others so the
    act-table-load pass keeps one set resident (no per-map Exp<->Ln table
    thrash).  Instance-level override; set ids keep matching act_info.json."""
    import bass_rust as _bass_rust
    from concourse.hw_specs import get_activation_tables

    keep = "natural_log_exp_and_others"
    af = mybir.ActivationFunctionType
    ours = {af.Exp, af.Ln, af.Copy, af.Identity}

    def patched_pass():
        has_activation = any(
            isinstance(i, mybir.InstActivation)
            for b in nc.main_func.blocks for i in b.instructions)
        if not has_activation:
            return
        tables = get_activation_tables(nc.m.arch)
        if keep in tables and ours <= set(tables[keep]):
            tables = {name: (fns if name == keep else set(fns) - ours)
                      for name, fns in tables.items()}
        _bass_rust.insert_act_table_loads(nc, list(tables.items()))

    nc.insert_act_table_loads = patched_pass


def _host_prep(x, w_q, w_k, w_v, w_head, in_gamma, in_beta, w_proj, b_proj):
    """Build the 8 per-core input maps (all fp32 numpy)."""
    x = np.asarray(x, dtype=np.float32)
    w_q = np.asarray(w_q, dtype=np.float32)
    w_k = np.asarray(w_k, dtype=np.float32)
    w_v = np.asarray(w_v, dtype=np.float32)
    w_head = np.asarray(w_head, dtype=np.float32)

    wqT = np.ascontiguousarray(w_q.T) * np.float32(SCALE)
    wkT = np.ascontiguousarray(w_k.T)
    # wp2[64*(s%2)+d, 512*(s//2)+c] = w_proj[c, 64*s+d]
    wpT_r = np.ascontiguousarray(
        np.asarray(w_proj, dtype=np.float32)
        .T.reshape(4, 2, 64, C).transpose(1, 2, 0, 3).reshape(128, 4 * C))

    in_maps = []
    p = np.arange(128)
    for core in range(NCORES):
        b = core // 2
        g0 = (core % 2) * GPC
        xc = np.ascontiguousarray(x[b].reshape(C, T))
        wvT = np.ascontiguousarray(w_v.T[:, g0 * HD:(g0 + GPC) * HD])
        al = np.empty((128, CC * GPC), dtype=np.float32)
        for cc in range(CC):
            for gi in range(GPC):
                al[:, cc * GPC + gi] = w_head[g0 + gi, cc * 2 + p // 64]
        in_maps.append({
            "x": xc, "wqT": wqT, "wkT": wkT, "wvT": wvT,
            "wpT": wpT_r, "alphas": al,
            "ones": np.ones((128, 128), dtype=np.float32),
        })
    return in_maps


def _host_finish(results, x, w_v, w_head, in_gamma, in_beta, w_proj, b_proj):
    in_gamma = np.asarray(in_gamma, dtype=np.float32)
    in_beta = np.asarray(in_beta, dtype=np.float32)
    w_proj = np.asarray(w_proj, dtype=np.float32)
    b_proj = np.asarray(b_proj, dtype=np.float32)
    w_v = np.asarray(w_v, dtype=np.float32)
    x = np.asarray(x, dtype=np.float32)

    # collapsed_wp[d, c] = sum_jh w_proj[c, jh*64+d]
    collapsed_wp = w_proj.reshape(C, 8, 64).sum(axis=1).T   # [64, C]
    out = np.empty((B, C, T), dtype=np.float32)
    for core in range(NCORES):
        b = core // 2
        g0 = (core % 2) * GPC
        dev = results[core]["out"]              # [512 i, 512 c]
        s2 = results[core]["s2"].sum(axis=1)    # [GPC]
        sv = w_v @ x[b].reshape(C, T).sum(axis=1)   # [C]
        for gi in range(GPC):
            g = g0 + gi
            var = s2[gi] / float(T * T) - MU * MU
            a = in_gamma[g] / np.sqrt(var + EPS)
            cs = in_beta[g] - a * MU
            bias2 = collapsed_wp.T @ sv[g * HD:(g + 1) * HD]   # [C]
            blk = dev[gi * 128:(gi + 1) * 128, :]              # [128 i, C]
            full = a * blk + (cs * bias2 + b_proj)[None, :]
            out[b, :, g * 128:(g + 1) * 128] = full.T
    return out.reshape(B, C, HH, WW)


def _run(inputs, trace=False, reps=1):
    nc = build_program(reps)
    in_maps = _host_prep(**inputs)
    res = run_bass_kernel_spmd(nc, in_maps, core_ids=list(range(NCORES)),
                               trace=trace)
    out = _host_finish(res.results, inputs["x"], inputs["w_v"],
                       inputs["w_head"], inputs["in_gamma"],
                       inputs["in_beta"], inputs["w_proj"], inputs["b_proj"])
    return out, res


def kernel(**inputs):
    out, _ = _run(inputs, trace=False)
    return out



# revision 25
# speedup vs baseline: 1.0277x; 1.0277x over previous
"""CMHSA (1x1-conv multi-head self-attention with a head-mixing 1x1 conv and
instance-norm on the attention maps) as a Trainium2 Bass kernel on 8
NeuronCores.

Reference math (B=4, C=512, T=1024, HEADS=8, hd=64):
  xf = x[b] as [C, T];  q/k/v = W @ xf;  per head h: S_h = q_h^T k_h * hd^-.5
  S'_g = sum_h w_head[g,h] S_h            (head-mixing 1x1 conv)
  A = softmax(S'_g, axis=-1)
  A = instnorm(A) * gamma_g + beta_g      (stats over the whole [T,T] map)
  out = (A @ v_g^T).view(b, t, c) @ w_proj.T + b_proj -> [B, C, H, W]

Transformations:
  * Head-mixing folds into Q: S'_g = (alpha_g . q)^T k with per-channel
    scale alpha_g[o] = w_head[g, o//64].  Each (b, g) map becomes fully
    independent -> 32 maps over 8 cores, 4 maps/core, zero collectives.
  * Logits are ~N(0,1): softmax without max-subtraction is safe.
  * Attention is computed transposed (S^T[T, q]) so the T (softmax) axis is
    the PE contraction axis; softmax row-sums come out of the AV matmul by
    appending 64 ones-columns to the stationary [v_g | 1]: PSUM rows 0-63 =
    v @ E, rows 64-127 = rowsum (pre-broadcast).  A second column-tiled
    matmul with an all-ones stationary reduces E^2 for the variance.
  * gamma/inv_std/beta/b_proj and the constant (beta - a*mu) * sum_T v term
    fold into a host epilogue given per-map sum_q sqsum/rowsum^2, which the
    device emits as a tiny second output.
  * The G map is written parity-split (g2[64*(s%2)+d, 128*(s//2)+i] =
    G[d, 8i+s]) so the projection contracts 128 partitions per matmul:
    4 matmuls/map instead of 8 while still realizing torch's
    .view(b, t, c) shuffle for free.
  * The 64 ones-columns of the AV stationary give the rowsum already
    broadcast across PSUM partitions 64-127 for free (PE cost is
    moving-rows only), so no separate broadcast matmul is needed: the
    reciprocal chain runs directly on those rows.  The sq stationary is
    sliced to 1 column (LDW is self-loading per matmul; narrower loads
    faster), and the static ones-columns of V' fill once outside the
    rep loop.
"""

import numpy as np

import concourse.bass as bass
import concourse.tile as tile
import concourse.mybir as mybir
from concourse import bacc
from concourse.bass_utils import run_bass_kernel_spmd

F32 = mybir.dt.float32
F32R = mybir.dt.float32r

B, C, HH, WW = 4, 512, 32, 32
T = HH * WW          # 1024
HEADS, HD = 8, 64
EPS = 1e-5
SCALE = HD ** -0.5   # 1/8
NCORES = 8
GPC = HEADS // 2     # 4 maps (g values) per core; 2 cores per batch
CC = C // 128        # 4 contraction chunks
TB = T // 128        # 8 T-blocks
MU = 1.0 / T

_prog_cache = {}


def build_program(reps=1):
    """Build + compile the SPMD Bass program (one NEFF, same for all cores).

    reps>1 repeats the whole compute body (for wall-clock timing via
    wall(reps=R) - wall(reps=1)); input loads run once."""
    if reps in _prog_cache:
        return _prog_cache[reps]

    nc = bacc.Bacc("TRN2", target_bir_lowering=False, debug=False,
                   num_devices=NCORES)

    x_d = nc.dram_tensor("x", [C, T], F32R, kind="ExternalInput")
    wq_d = nc.dram_tensor("wqT", [C, C], F32R, kind="ExternalInput")
    wk_d = nc.dram_tensor("wkT", [C, C], F32R, kind="ExternalInput")
    wv_d = nc.dram_tensor("wvT", [C, GPC * HD], F32R, kind="ExternalInput")
    wp_d = nc.dram_tensor("wpT", [128, 4 * C], F32R, kind="ExternalInput")
    al_d = nc.dram_tensor("alphas", [128, CC * GPC], F32, kind="ExternalInput")
    on_d = nc.dram_tensor("ones", [128, 128], F32R, kind="ExternalInput")
    out_d = nc.dram_tensor("out", [GPC * 128, C], F32, kind="ExternalOutput")
    s2_d = nc.dram_tensor("s2", [GPC, 2], F32, kind="ExternalOutput")

    with tile.TileContext(nc) as tc:
        with (
            tc.tile_pool(name="persist", bufs=1) as persist,
            tc.tile_pool(name="qg", bufs=2) as qg_pool,
            tc.tile_pool(name="e", bufs=3) as e_pool,
            tc.tile_pool(name="e2", bufs=3) as e2_pool,
            tc.tile_pool(name="g", bufs=2) as g_pool,
            tc.tile_pool(name="st", bufs=2) as st_pool,
            tc.tile_pool(name="qkps", bufs=2, space="PSUM") as qk_ps,
            tc.tile_pool(name="avps", bufs=2, space="PSUM") as av_ps,
        ):
            # ---------------- load inputs ----------------
            x_sb = persist.tile([128, CC * T], F32R)   # x[c,t]; chunk cc at cols cc*T
            for cc in range(CC):
                for th in range(2):
                    nc.sync.dma_start(
                        x_sb[:, cc * T + th * 512:cc * T + (th + 1) * 512],
                        x_d[cc * 128:(cc + 1) * 128,
                            th * 512:(th + 1) * 512])
            wq_sb = persist.tile([128, CC * C], F32R)  # w_q.T/8; chunk cc at cols cc*C
            wk_sb = persist.tile([128, CC * C], F32R)
            for w_sb, w_d in ((wq_sb, wq_d), (wk_sb, wk_d)):
                for cc in range(CC):
                    nc.sync.dma_start(w_sb[:, cc * C:(cc + 1) * C],
                                      w_d[cc * 128:(cc + 1) * 128, :])
            wv_sb = persist.tile([128, CC * GPC * HD], F32R)  # this core's v heads
            for cc in range(CC):
                nc.sync.dma_start(
                    wv_sb[:, cc * GPC * HD:(cc + 1) * GPC * HD],
                    wv_d[cc * 128:(cc + 1) * 128, :])
            # wp2[64*(s%2)+d, 512*(s//2)+c] = w_proj[c, 64*s+d]; pairs the 8
            # stride-8 projection chunks into 4 matmuls of 128-deep contraction
            wp_sb = persist.tile([128, 4 * C], F32R)
            nc.sync.dma_start(wp_sb[:], wp_d[:])
            al_sb = persist.tile([128, CC * GPC], F32)
            nc.sync.dma_start(al_sb[:], al_d[:])
            ones_sb = persist.tile([128, 128], F32R)
            nc.sync.dma_start(ones_sb[:], on_d[:])

            # V' = [v-slices | ones] layout; the ones columns are static, so
            # fill them once outside the rep loop.
            # vp block tb (512 cols): [gi*128, gi*128+64) = V^T[:, gi*64..]
            #                         [gi*128+64, gi*128+128) = ones
            vp_sb = persist.tile([128, TB * 512], F32R)
            vp_v = vp_sb[:].rearrange("p (t g k) -> p t g k", g=GPC, k=128)
            for tb in range(TB):
                nc.sync.dma_start(
                    vp_v[:, tb, :, 64:128],
                    bass.AP(tensor=on_d, offset=0,
                            ap=[[128, 128], [0, GPC], [1, 64]]))

            for _rep in range(reps):
                # ---------------- Q, K = W @ x ----------------
                q_sb = persist.tile([128, CC * T], F32R)   # Q[o,t]; chunk ob at cols ob*T
                k_sb = persist.tile([128, CC * T], F32R)
                for w_sb, dst in ((wq_sb, q_sb), (wk_sb, k_sb)):
                    for ob in range(4):
                        ps = qk_ps.tile([128, 1024], F32, tag="mmps", name="qkv_ps")
                        for th in range(2):
                            for cc in range(CC):
                                nc.tensor.matmul(
                                    ps[:, th * 512:(th + 1) * 512],
                                    (w_sb[:, cc * C + ob * 128:
                                            cc * C + (ob + 1) * 128]),
                                    (x_sb[:, cc * T + th * 512:
                                            cc * T + th * 512 + 512]),
                                    start=(cc == 0), stop=(cc == CC - 1))
                        nc.scalar.copy(dst[:, ob * T:(ob + 1) * T], ps[:])

                # ---------------- V' v-slices per T-block ----------------
                for tb in range(TB):
                    ps = qk_ps.tile([128, 1024], F32, tag="mmps", name="vt_ps")
                    for cc in range(CC):
                        nc.tensor.matmul(
                            ps[:, 0:GPC * HD],
                            (x_sb[:, cc * T + tb * 128:cc * T + (tb + 1) * 128]),
                            (wv_sb[:, cc * GPC * HD:(cc + 1) * GPC * HD]),
                            start=(cc == 0), stop=(cc == CC - 1))
                    nc.vector.tensor_copy(
                        vp_v[:, tb, :, 0:64],
                        ps[:, 0:GPC * HD].rearrange("p (g k) -> p g k", k=64))

                # ---------------- per-map pipeline ----------------
                # A single emission FIFO keeps every non-QK chunk of work
                # (AV/sqsum matmuls, epilogue, projection) trailing ~2 steps
                # behind the QK stream, across map boundaries, so the PE's
                # static order always has QK matmuls to chew while ACT (exp)
                # and the rowsum-reciprocal chain catch up.

                from collections import deque
                todo = deque()

                def emit_qg(gi):
                    qg_sb = qg_pool.tile([128, CC * T], F32R, tag="qg",
                                         name=f"qg{gi}")
                    for cc in range(CC):
                        nc.vector.tensor_scalar_mul(
                            qg_sb[:, cc * T:(cc + 1) * T],
                            q_sb[:, cc * T:(cc + 1) * T],
                            al_sb[:, cc * GPC + gi:cc * GPC + gi + 1])
                    return qg_sb

                def emit_avsq(gi, tb, avs, sqs, e_t, e2_t):
                    # Full [v | ones x64] stationary: PSUM rows 64-127 come
                    # out as the rowsum already broadcast across 64
                    # partitions (PE cost is moving-rows only), which makes
                    # a separate broadcast matmul unnecessary.  The sq
                    # stationary stays 1 column (only row 0 is read).
                    for qh in range(2):
                        nc.tensor.matmul(
                            avs[qh][:, :],
                            vp_sb[:, tb * 512 + gi * 128:
                                  tb * 512 + (gi + 1) * 128],
                            e_t[:, qh * 512:(qh + 1) * 512],
                            start=(tb == 0), stop=(tb == TB - 1))
                        nc.tensor.matmul(
                            sqs[qh][0:1, :],
                            ones_sb[:, 0:1],
                            e2_t[:, qh * 512:(qh + 1) * 512],
                            start=(tb == 0), stop=(tb == TB - 1))

                def emit_epilogue(gi, avs, sqs, g2_sb):
                    # Copy av PSUM to SBUF immediately (releases the
                    # accumulator banks before the reciprocal chain).
                    s2_t = st_pool.tile([1, 2], F32, tag="s2_t", name="s2_t")
                    avc = []
                    for qh in range(2):
                        a_sb = st_pool.tile([128, 512], F32, tag="avc",
                                            name="a_sb", bufs=3)
                        nc.vector.tensor_copy(a_sb[0:64, :],
                                              avs[qh][0:64, :])
                        avc.append(a_sb)
                    for qh in range(2):
                        # r = 1/rowsum via exp(-ln .) on ACT (the pinned
                        # table set has both), reading the 64 pre-broadcast
                        # rowsum rows straight from PSUM and shifting them
                        # to base partition 0 (single-input op, so the
                        # SBUF-SBUF base-alignment rule doesn't apply).
                        lnr = st_pool.tile([128, 512], F32, tag="lnr",
                                           name="lnr")
                        nc.scalar.activation(lnr[0:64, :],
                                             avs[qh][64:128, :],
                                             mybir.ActivationFunctionType.Ln)
                        rrow = st_pool.tile([128, 512], F32, tag="rrow",
                                            name="rrow")
                        nc.scalar.activation(rrow[0:64, :], lnr[0:64, :],
                                             mybir.ActivationFunctionType.Exp,
                                             scale=-1.0)
                        # G = (v@E) * r, written parity-split so the
                        # projection can contract 128 partitions at a time:
                        # g2[64*(s%2)+d, 128*(s//2)+i] = G[d, q=8i+s]
                        av_r = avc[qh][:].rearrange("p (i j k) -> p k j i",
                                                    j=4, k=2)
                        rb_r = rrow[:].rearrange("p (i j k) -> p k j i",
                                                 j=4, k=2)
                        g2_r = g2_sb[:].rearrange("p (j i) -> p j i", i=128)
                        for par in range(2):
                            nc.vector.tensor_tensor(
                                g2_r[64 * par:64 * par + 64, :,
                                     qh * 64:qh * 64 + 64],
                                av_r[0:64, par, :, :],
                                rb_r[0:64, par, :, :],
                                mybir.AluOpType.mult)
                        # s2[qh] = sum_q (sum_T E^2) * r^2, reading sum_T E^2
                        # straight off PSUM row 0 (all sq rows identical).
                        r2 = st_pool.tile([128, 512], F32, tag="r2",
                                          name="r2")
                        nc.vector.tensor_mul(r2[0:1, :], rrow[0:1, :],
                                             rrow[0:1, :])
                        u = st_pool.tile([128, 512], F32, tag="u", name="u")
                        nc.vector.tensor_tensor(u[0:1, :], r2[0:1, :],
                                                sqs[qh][0:1, :],
                                                mybir.AluOpType.mult)
                        nc.vector.reduce_sum(s2_t[0:1, qh:qh + 1],
                                             u[0:1, :],
                                             axis=mybir.AxisListType.X)
                    nc.sync.dma_start(s2_d[gi:gi + 1, :], s2_t[0:1, :])

                def emit_proj(gi, g2_sb):
                    # out^T[i,c] = sum_s sum_d G[d, 8i+s] * wp[d, s*512+c],
                    # two s-parities stacked per 128-deep contraction chunk
                    p_ps = av_ps.tile([128, 512], F32, tag="av", name="p_ps")
                    for j2 in range(4):
                        nc.tensor.matmul(p_ps[:],
                                         g2_sb[:, j2 * 128:(j2 + 1) * 128],
                                         wp_sb[:, j2 * C:(j2 + 1) * C],
                                         start=(j2 == 0), stop=(j2 == 3))
                    stage = st_pool.tile([128, 512], F32, tag="stage",
                                         name="stage", bufs=2)
                    nc.scalar.copy(stage[:], p_ps[:])
                    nc.sync.dma_start(out_d[gi * 128:(gi + 1) * 128, :],
                                      stage[:])

                qg_next = emit_qg(0)
                for gi in range(GPC):
                    qg_sb = qg_next
                    avs = tuple(av_ps.tile([128, 512], F32, tag="av",
                                           name=f"av{qh}") for qh in range(2))
                    sqs = tuple(av_ps.tile([128, 512], F32, tag="sq",
                                           name=f"sq{qh}") for qh in range(2))

                    for tb in range(TB):
                        s_ps = qk_ps.tile([128, 1024], F32, tag="mmps",
                                          name="s_ps")
                        for qh in range(2):
                            for oc in range(CC):
                                nc.tensor.matmul(
                                    s_ps[:, qh * 512:(qh + 1) * 512],
                                    k_sb[:, oc * T + tb * 128:
                                         oc * T + (tb + 1) * 128],
                                    qg_sb[:, oc * T + qh * 512:
                                          oc * T + qh * 512 + 512],
                                    start=(oc == 0), stop=(oc == CC - 1))
                        e_t = e_pool.tile([128, 1024], F32R)
                        nc.scalar.activation(e_t[:], s_ps[:],
                                             mybir.ActivationFunctionType.Exp)
                        e2_t = e2_pool.tile([128, 1024], F32R)
                        if tb % 2 == 0:
                            nc.vector.tensor_mul(e2_t[:], e_t[:], e_t[:])
                        else:
                            nc.scalar.activation(
                                e2_t[:], s_ps[:],
                                mybir.ActivationFunctionType.Exp, scale=2.0)
                        todo.append(lambda gi=gi, tb=tb, a=avs, s=sqs,
                                    e=e_t, e2=e2_t:
                                    emit_avsq(gi, tb, a, s, e, e2))
                        if tb == 4 and gi + 1 < GPC:
                            qg_next = emit_qg(gi + 1)
                        while len(todo) > 2:
                            todo.popleft()()
                    g2_sb = g_pool.tile([128, 512], F32R)
                    todo.append(lambda gi=gi, a=avs, s=sqs, g=g2_sb:
                                emit_epilogue(gi, a, s, g))
                    todo.append(lambda gi=gi, g=g2_sb: emit_proj(gi, g))
                while todo:
                    todo.popleft()()

    _pin_act_table(nc)
    nc.compile()
    _prog_cache[reps] = nc
    return nc


def _pin_act_table(nc):
    """Make Exp/Ln/Copy resolvable only via natural_log_exp_and_others so the
    act-table-load pass keeps one set resident (no per-map Exp<->Ln table
    thrash).  Instance-level override; set ids keep matching act_info.json."""
    import bass_rust as _bass_rust
    from concourse.hw_specs import get_activation_tables

    keep = "natural_log_exp_and_others"
    af = mybir.ActivationFunctionType
    ours = {af.Exp, af.Ln, af.Copy, af.Identity}

    def patched_pass():
        has_activation = any(
            isinstance(i, mybir.InstActivation)
            for b in nc.main_func.blocks for i in b.instructions)
        if not has_activation:
            return
        tables = get_activation_tables(nc.m.arch)
        if keep in tables and ours <= set(tables[keep]):
            tables = {name: (fns if name == keep else set(fns) - ours)
                      for name, fns in tables.items()}
        _bass_rust.insert_act_table_loads(nc, list(tables.items()))

    nc.insert_act_table_loads = patched_pass


def _host_prep(x, w_q, w_k, w_v, w_head, in_gamma, in_beta, w_proj, b_proj):
    """Build the 8 per-core input maps (all fp32 numpy)."""
    x = np.asarray(x, dtype=np.float32)
    w_q = np.asarray(w_q, dtype=np.float32)
    w_k = np.asarray(w_k, dtype=np.float32)
    w_v = np.asarray(w_v, dtype=np.float32)
    w_head = np.asarray(w_head, dtype=np.float32)

    wqT = np.ascontiguousarray(w_q.T) * np.float32(SCALE)
    wkT = np.ascontiguousarray(w_k.T)
    # wp2[64*(s%2)+d, 512*(s//2)+c] = w_proj[c, 64*s+d]
    wpT_r = np.ascontiguousarray(
        np.asarray(w_proj, dtype=np.float32)
        .T.reshape(4, 2, 64, C).transpose(1, 2, 0, 3).reshape(128, 4 * C))

    in_maps = []
    p = np.arange(128)
    for core in range(NCORES):
        b = core // 2
        g0 = (core % 2) * GPC
        xc = np.ascontiguousarray(x[b].reshape(C, T))
        wvT = np.ascontiguousarray(w_v.T[:, g0 * HD:(g0 + GPC) * HD])
        al = np.empty((128, CC * GPC), dtype=np.float32)
        for cc in range(CC):
            for gi in range(GPC):
                al[:, cc * GPC + gi] = w_head[g0 + gi, cc * 2 + p // 64]
        in_maps.append({
            "x": xc, "wqT": wqT, "wkT": wkT, "wvT": wvT,
            "wpT": wpT_r, "alphas": al,
            "ones": np.ones((128, 128), dtype=np.float32),
        })
    return in_maps


def _host_finish(results, x, w_v, w_head, in_gamma, in_beta, w_proj, b_proj):
    in_gamma = np.asarray(in_gamma, dtype=np.float32)
    in_beta = np.asarray(in_beta, dtype=np.float32)
    w_proj = np.asarray(w_proj, dtype=np.float32)
    b_proj = np.asarray(b_proj, dtype=np.float32)
    w_v = np.asarray(w_v, dtype=np.float32)
    x = np.asarray(x, dtype=np.float32)

    # collapsed_wp[d, c] = sum_jh w_proj[c, jh*64+d]
    collapsed_wp = w_proj.reshape(C, 8, 64).sum(axis=1).T   # [64, C]
    out = np.empty((B, C, T), dtype=np.float32)
    for core in range(NCORES):
        b = core // 2
        g0 = (core % 2) * GPC
        dev = results[core]["out"]              # [512 i, 512 c]
        s2 = results[core]["s2"].sum(axis=1)    # [GPC]
        sv = w_v @ x[b].reshape(C, T).sum(axis=1)   # [C]
        for gi in range(GPC):
            g = g0 + gi
            var = s2[gi] / float(T * T) - MU * MU
            a = in_gamma[g] / np.sqrt(var + EPS)
            cs = in_beta[g] - a * MU
            bias2 = collapsed_wp.T @ sv[g * HD:(g + 1) * HD]   # [C]
            blk = dev[gi * 128:(gi + 1) * 128, :]              # [128 i, C]
            full = a * blk + (cs * bias2 + b_proj)[None, :]
            out[b, :, g * 128:(g + 1) * 128] = full.T
    return out.reshape(B, C, HH, WW)


def _run(inputs, trace=False, reps=1):
    nc = build_program(reps)
    in_maps = _host_prep(**inputs)
    res = run_bass_kernel_spmd(nc, in_maps, core_ids=list(range(NCORES)),
                               trace=trace)
    out = _host_finish(res.results, inputs["x"], inputs["w_v"],
                       inputs["w_head"], inputs["in_gamma"],
                       inputs["in_beta"], inputs["w_proj"], inputs["b_proj"])
    return out, res


def kernel(**inputs):
    out, _ = _run(inputs, trace=False)
    return out



# revision 28
# speedup vs baseline: 1.1182x; 1.0880x over previous
"""CMHSA (1x1-conv multi-head self-attention with a head-mixing 1x1 conv and
instance-norm on the attention maps) as a Trainium2 Bass kernel on 8
NeuronCores.

Reference math (B=4, C=512, T=1024, HEADS=8, hd=64):
  xf = x[b] as [C, T];  q/k/v = W @ xf;  per head h: S_h = q_h^T k_h * hd^-.5
  S'_g = sum_h w_head[g,h] S_h            (head-mixing 1x1 conv)
  A = softmax(S'_g, axis=-1)
  A = instnorm(A) * gamma_g + beta_g      (stats over the whole [T,T] map)
  out = (A @ v_g^T).view(b, t, c) @ w_proj.T + b_proj -> [B, C, H, W]

Transformations:
  * Head-mixing folds into Q: S'_g = (alpha_g . q)^T k with per-channel
    scale alpha_g[o] = w_head[g, o//64].  Each (b, g) map becomes fully
    independent -> 32 maps over 8 cores, 4 maps/core, zero collectives.
  * Logits are ~N(0,1): softmax without max-subtraction is safe.
  * Attention is computed transposed (S^T[T, q]) so the T (softmax) axis is
    the PE contraction axis; softmax row-sums come out of the AV matmul by
    appending 64 ones-columns to the stationary [v_g | 1]: PSUM rows 0-63 =
    v @ E, rows 64-127 = rowsum (pre-broadcast).  A second column-tiled
    matmul with an all-ones stationary reduces E^2 for the variance.
  * gamma/inv_std/beta/b_proj and the constant (beta - a*mu) * sum_T v term
    fold into a host epilogue given per-map sum_q sqsum/rowsum^2, which the
    device emits as a tiny second output.
  * The G map is written parity-split (g2[64*(s%2)+d, 128*(s//2)+i] =
    G[d, 8i+s]) so the projection contracts 128 partitions per matmul:
    4 matmuls/map instead of 8 while still realizing torch's
    .view(b, t, c) shuffle for free.
  * The 64 ones-columns of the AV stationary give the rowsum already
    broadcast across PSUM partitions 64-127 for free (PE cost is
    moving-rows only), so no separate broadcast matmul is needed: the
    reciprocal chain runs directly on those rows.  The sq stationary is
    sliced to 1 column (LDW is self-loading per matmul; narrower loads
    faster), and the static ones-columns of V' fill once outside the
    rep loop.
"""

import numpy as np

import concourse.bass as bass
import concourse.tile as tile
import concourse.mybir as mybir
from concourse import bacc
from concourse.bass_utils import run_bass_kernel_spmd

F32 = mybir.dt.float32
F32R = mybir.dt.float32r

B, C, HH, WW = 4, 512, 32, 32
T = HH * WW          # 1024
HEADS, HD = 8, 64
EPS = 1e-5
SCALE = HD ** -0.5   # 1/8
NCORES = 8
GPC = HEADS // 2     # 4 maps (g values) per core; 2 cores per batch
CC = C // 128        # 4 contraction chunks
TB = T // 128        # 8 T-blocks
MU = 1.0 / T

_prog_cache = {}


def build_program(reps=1):
    """Build + compile the SPMD Bass program (one NEFF, same for all cores).

    reps>1 repeats the whole compute body (for wall-clock timing via
    wall(reps=R) - wall(reps=1)); input loads run once."""
    if reps in _prog_cache:
        return _prog_cache[reps]

    nc = bacc.Bacc("TRN2", target_bir_lowering=False, debug=False,
                   num_devices=NCORES)

    x_d = nc.dram_tensor("x", [C, T], F32R, kind="ExternalInput")
    wq_d = nc.dram_tensor("wqT", [C, C], F32R, kind="ExternalInput")
    wk_d = nc.dram_tensor("wkT", [C, C], F32R, kind="ExternalInput")
    wv_d = nc.dram_tensor("wvT", [C, GPC * HD], F32R, kind="ExternalInput")
    wp_d = nc.dram_tensor("wpT", [128, 4 * C], F32R, kind="ExternalInput")
    al_d = nc.dram_tensor("alphas", [128, CC * GPC], F32, kind="ExternalInput")
    on_d = nc.dram_tensor("ones", [128, 128], F32R, kind="ExternalInput")
    out_d = nc.dram_tensor("out", [GPC * 128, C], F32, kind="ExternalOutput")
    s2_d = nc.dram_tensor("s2", [GPC, 2], F32, kind="ExternalOutput")

    with tile.TileContext(nc) as tc:
        with (
            tc.tile_pool(name="persist", bufs=1) as persist,
            tc.tile_pool(name="qg", bufs=2) as qg_pool,
            tc.tile_pool(name="e", bufs=3) as e_pool,
            tc.tile_pool(name="e2", bufs=3) as e2_pool,
            tc.tile_pool(name="g", bufs=2) as g_pool,
            tc.tile_pool(name="st", bufs=2) as st_pool,
            tc.tile_pool(name="qkps", bufs=2, space="PSUM") as qk_ps,
            tc.tile_pool(name="avps", bufs=2, space="PSUM") as av_ps,
        ):
            # ---------------- load inputs ----------------
            x_sb = persist.tile([128, CC * T], F32R)   # x[c,t]; chunk cc at cols cc*T
            for cc in range(CC):
                for th in range(2):
                    nc.sync.dma_start(
                        x_sb[:, cc * T + th * 512:cc * T + (th + 1) * 512],
                        x_d[cc * 128:(cc + 1) * 128,
                            th * 512:(th + 1) * 512])
            wq_sb = persist.tile([128, CC * C], F32R)  # w_q.T/8; chunk cc at cols cc*C
            wk_sb = persist.tile([128, CC * C], F32R)
            for w_sb, w_d in ((wq_sb, wq_d), (wk_sb, wk_d)):
                for cc in range(CC):
                    nc.sync.dma_start(w_sb[:, cc * C:(cc + 1) * C],
                                      w_d[cc * 128:(cc + 1) * 128, :])
            wv_sb = persist.tile([128, CC * GPC * HD], F32R)  # this core's v heads
            for cc in range(CC):
                nc.sync.dma_start(
                    wv_sb[:, cc * GPC * HD:(cc + 1) * GPC * HD],
                    wv_d[cc * 128:(cc + 1) * 128, :])
            # wp2[64*(s%2)+d, 512*(s//2)+c] = w_proj[c, 64*s+d]; pairs the 8
            # stride-8 projection chunks into 4 matmuls of 128-deep contraction
            wp_sb = persist.tile([128, 4 * C], F32R)
            nc.sync.dma_start(wp_sb[:], wp_d[:])
            al_sb = persist.tile([128, CC * GPC], F32)
            nc.sync.dma_start(al_sb[:], al_d[:])
            ones_sb = persist.tile([128, 128], F32R)
            nc.sync.dma_start(ones_sb[:], on_d[:])

            # V' = [v-slices | ones] layout; the ones columns are static, so
            # fill them once outside the rep loop.
            # vp block tb (512 cols): [gi*128, gi*128+64) = V^T[:, gi*64..]
            #                         [gi*128+64, gi*128+128) = ones
            vp_sb = persist.tile([128, TB * 512], F32R)
            vp_v = vp_sb[:].rearrange("p (t g k) -> p t g k", g=GPC, k=128)
            for tb in range(TB):
                nc.sync.dma_start(
                    vp_v[:, tb, :, 64:128],
                    bass.AP(tensor=on_d, offset=0,
                            ap=[[128, 128], [0, GPC], [1, 64]]))

            for _rep in range(reps):
                # ---------------- Q, K = W @ x ----------------
                q_sb = persist.tile([128, CC * T], F32R)   # Q[o,t]; chunk ob at cols ob*T
                k_sb = persist.tile([128, CC * T], F32R)
                for w_sb, dst in ((wq_sb, q_sb), (wk_sb, k_sb)):
                    for ob in range(4):
                        ps = qk_ps.tile([128, 1024], F32, tag="mmps", name="qkv_ps")
                        for th in range(2):
                            for cc in range(CC):
                                nc.tensor.matmul(
                                    ps[:, th * 512:(th + 1) * 512],
                                    (w_sb[:, cc * C + ob * 128:
                                            cc * C + (ob + 1) * 128]),
                                    (x_sb[:, cc * T + th * 512:
                                            cc * T + th * 512 + 512]),
                                    start=(cc == 0), stop=(cc == CC - 1))
                        nc.scalar.copy(dst[:, ob * T:(ob + 1) * T], ps[:])

                # ---------------- V' v-slices per T-block ----------------
                for tb in range(TB):
                    ps = qk_ps.tile([128, 1024], F32, tag="mmps", name="vt_ps")
                    for cc in range(CC):
                        nc.tensor.matmul(
                            ps[:, 0:GPC * HD],
                            (x_sb[:, cc * T + tb * 128:cc * T + (tb + 1) * 128]),
                            (wv_sb[:, cc * GPC * HD:(cc + 1) * GPC * HD]),
                            start=(cc == 0), stop=(cc == CC - 1))
                    nc.vector.tensor_copy(
                        vp_v[:, tb, :, 0:64],
                        ps[:, 0:GPC * HD].rearrange("p (g k) -> p g k", k=64))

                # ---------------- per-map pipeline ----------------
                # A single emission FIFO keeps every non-QK chunk of work
                # (AV/sqsum matmuls, epilogue, projection) trailing ~2 steps
                # behind the QK stream, across map boundaries, so the PE's
                # static order always has QK matmuls to chew while ACT (exp)
                # and the rowsum-reciprocal chain catch up.

                from collections import deque
                todo = deque()

                def emit_qg(gi):
                    qg_sb = qg_pool.tile([128, CC * T], F32R, tag="qg",
                                         name=f"qg{gi}")
                    for cc in range(CC):
                        nc.vector.tensor_scalar_mul(
                            qg_sb[:, cc * T:(cc + 1) * T],
                            q_sb[:, cc * T:(cc + 1) * T],
                            al_sb[:, cc * GPC + gi:cc * GPC + gi + 1])
                    return qg_sb

                def emit_avsq(gi, tb, avs, sqs, e_t, e2_t):
                    # Full [v | ones x64] stationary: PSUM rows 64-127 come
                    # out as the rowsum already broadcast across 64
                    # partitions (PE cost is moving-rows only), which makes
                    # a separate broadcast matmul unnecessary.  The sq
                    # stationary stays 1 column (only row 0 is read).
                    for qh in range(2):
                        nc.tensor.matmul(
                            avs[qh][:, :],
                            vp_sb[:, tb * 512 + gi * 128:
                                  tb * 512 + (gi + 1) * 128],
                            e_t[:, qh * 512:(qh + 1) * 512],
                            start=(tb == 0), stop=(tb == TB - 1))
                        nc.tensor.matmul(
                            sqs[qh][0:1, :],
                            ones_sb[:, 0:1],
                            e2_t[:, qh * 512:(qh + 1) * 512],
                            start=(tb == 0), stop=(tb == TB - 1))

                def emit_epilogue(gi, avs, sqs, g2_sb):
                    # Copy av PSUM to SBUF immediately (releases the
                    # accumulator banks before the reciprocal chain).
                    s2_t = st_pool.tile([1, 2], F32, tag="s2_t", name="s2_t")
                    avc = []
                    for qh in range(2):
                        a_sb = st_pool.tile([128, 512], F32, tag="avc",
                                            name="a_sb", bufs=3)
                        nc.vector.tensor_copy(a_sb[0:64, :],
                                              avs[qh][0:64, :])
                        avc.append(a_sb)
                    for qh in range(2):
                        # r = 1/rowsum via exp(-ln .) on ACT (the pinned
                        # table set has both), reading the 64 pre-broadcast
                        # rowsum rows straight from PSUM and shifting them
                        # to base partition 0 (single-input op, so the
                        # SBUF-SBUF base-alignment rule doesn't apply).
                        lnr = st_pool.tile([128, 512], F32, tag="lnr",
                                           name="lnr")
                        nc.scalar.activation(lnr[0:64, :],
                                             avs[qh][64:128, :],
                                             mybir.ActivationFunctionType.Ln)
                        rrow = st_pool.tile([128, 512], F32, tag="rrow",
                                            name="rrow")
                        nc.scalar.activation(rrow[0:64, :], lnr[0:64, :],
                                             mybir.ActivationFunctionType.Exp,
                                             scale=-1.0)
                        # G = (v@E) * r, written parity-split so the
                        # projection can contract 128 partitions at a time:
                        # g2[64*(s%2)+d, 128*(s//2)+i] = G[d, q=8i+s]
                        av_r = avc[qh][:].rearrange("p (i j k) -> p k j i",
                                                    j=4, k=2)
                        rb_r = rrow[:].rearrange("p (i j k) -> p k j i",
                                                 j=4, k=2)
                        g2_r = g2_sb[:].rearrange("p (j i) -> p j i", i=128)
                        for par in range(2):
                            nc.vector.tensor_tensor(
                                g2_r[64 * par:64 * par + 64, :,
                                     qh * 64:qh * 64 + 64],
                                av_r[0:64, par, :, :],
                                rb_r[0:64, par, :, :],
                                mybir.AluOpType.mult)
                        # s2[qh] = sum_q (sum_T E^2) * r^2, reading sum_T E^2
                        # straight off PSUM row 0 (all sq rows identical).
                        r2 = st_pool.tile([128, 512], F32, tag="r2",
                                          name="r2")
                        nc.vector.tensor_mul(r2[0:1, :], rrow[0:1, :],
                                             rrow[0:1, :])
                        u = st_pool.tile([128, 512], F32, tag="u", name="u")
                        nc.vector.tensor_tensor(u[0:1, :], r2[0:1, :],
                                                sqs[qh][0:1, :],
                                                mybir.AluOpType.mult)
                        nc.vector.reduce_sum(s2_t[0:1, qh:qh + 1],
                                             u[0:1, :],
                                             axis=mybir.AxisListType.X)
                    nc.sync.dma_start(s2_d[gi:gi + 1, :], s2_t[0:1, :])

                def emit_proj(gi, g2_sb):
                    # out^T[i,c] = sum_s sum_d G[d, 8i+s] * wp[d, s*512+c],
                    # two s-parities stacked per 128-deep contraction chunk.
                    # p_ps rides the "sq" ring (sqs are consumed by then) so
                    # the "av" slots recycle a full map apart -- the next
                    # map's first AV matmul would otherwise stall on this
                    # map's epilogue still reading avs PSUM.
                    p_ps = av_ps.tile([128, 512], F32, tag="sq", name="p_ps")
                    for j2 in range(4):
                        nc.tensor.matmul(p_ps[:],
                                         g2_sb[:, j2 * 128:(j2 + 1) * 128],
                                         wp_sb[:, j2 * C:(j2 + 1) * C],
                                         start=(j2 == 0), stop=(j2 == 3))
                    stage = st_pool.tile([128, 512], F32, tag="stage",
                                         name="stage", bufs=2)
                    nc.scalar.copy(stage[:], p_ps[:])
                    nc.sync.dma_start(out_d[gi * 128:(gi + 1) * 128, :],
                                      stage[:])

                qg_next = emit_qg(0)
                for gi in range(GPC):
                    qg_sb = qg_next
                    avs = tuple(av_ps.tile([128, 512], F32, tag="av",
                                           name=f"av{qh}") for qh in range(2))
                    sqs = tuple(av_ps.tile([128, 512], F32, tag="sq",
                                           name=f"sq{qh}") for qh in range(2))

                    for tb in range(TB):
                        s_ps = qk_ps.tile([128, 1024], F32, tag="mmps",
                                          name="s_ps")
                        for qh in range(2):
                            for oc in range(CC):
                                nc.tensor.matmul(
                                    s_ps[:, qh * 512:(qh + 1) * 512],
                                    k_sb[:, oc * T + tb * 128:
                                         oc * T + (tb + 1) * 128],
                                    qg_sb[:, oc * T + qh * 512:
                                          oc * T + qh * 512 + 512],
                                    start=(oc == 0), stop=(oc == CC - 1))
                        e_t = e_pool.tile([128, 1024], F32R)
                        nc.scalar.activation(e_t[:], s_ps[:],
                                             mybir.ActivationFunctionType.Exp)
                        e2_t = e2_pool.tile([128, 1024], F32R)
                        if tb % 2 == 0:
                            nc.vector.tensor_mul(e2_t[:], e_t[:], e_t[:])
                        else:
                            nc.scalar.activation(
                                e2_t[:], s_ps[:],
                                mybir.ActivationFunctionType.Exp, scale=2.0)
                        todo.append(lambda gi=gi, tb=tb, a=avs, s=sqs,
                                    e=e_t, e2=e2_t:
                                    emit_avsq(gi, tb, a, s, e, e2))
                        if tb == 4 and gi + 1 < GPC:
                            qg_next = emit_qg(gi + 1)
                        while len(todo) > 2:
                            todo.popleft()()
                    g2_sb = g_pool.tile([128, 512], F32R)
                    todo.append(lambda gi=gi, a=avs, s=sqs, g=g2_sb:
                                emit_epilogue(gi, a, s, g))
                    todo.append(lambda gi=gi, g=g2_sb: emit_proj(gi, g))
                while todo:
                    todo.popleft()()

    _pin_act_table(nc)
    nc.compile()
    _prog_cache[reps] = nc
    return nc


def _pin_act_table(nc):
    """Make Exp/Ln/Copy resolvable only via natural_log_exp_and_others so the
    act-table-load pass keeps one set resident (no per-map Exp<->Ln table
    thrash).  Instance-level override; set ids keep matching act_info.json."""
    import bass_rust as _bass_rust
    from concourse.hw_specs import get_activation_tables

    keep = "natural_log_exp_and_others"
    af = mybir.ActivationFunctionType
    ours = {af.Exp, af.Ln, af.Copy, af.Identity}

    def patched_pass():
        has_activation = any(
            isinstance(i, mybir.InstActivation)
            for b in nc.main_func.blocks for i in b.instructions)
        if not has_activation:
            return
        tables = get_activation_tables(nc.m.arch)
        if keep in tables and ours <= set(tables[keep]):
            tables = {name: (fns if name == keep else set(fns) - ours)
                      for name, fns in tables.items()}
        _bass_rust.insert_act_table_loads(nc, list(tables.items()))

    nc.insert_act_table_loads = patched_pass


def _host_prep(x, w_q, w_k, w_v, w_head, in_gamma, in_beta, w_proj, b_proj):
    """Build the 8 per-core input maps (all fp32 numpy)."""
    x = np.asarray(x, dtype=np.float32)
    w_q = np.asarray(w_q, dtype=np.float32)
    w_k = np.asarray(w_k, dtype=np.float32)
    w_v = np.asarray(w_v, dtype=np.float32)
    w_head = np.asarray(w_head, dtype=np.float32)

    wqT = np.ascontiguousarray(w_q.T) * np.float32(SCALE)
    wkT = np.ascontiguousarray(w_k.T)
    # wp2[64*(s%2)+d, 512*(s//2)+c] = w_proj[c, 64*s+d]
    wpT_r = np.ascontiguousarray(
        np.asarray(w_proj, dtype=np.float32)
        .T.reshape(4, 2, 64, C).transpose(1, 2, 0, 3).reshape(128, 4 * C))

    in_maps = []
    p = np.arange(128)
    for core in range(NCORES):
        b = core // 2
        g0 = (core % 2) * GPC
        xc = np.ascontiguousarray(x[b].reshape(C, T))
        wvT = np.ascontiguousarray(w_v.T[:, g0 * HD:(g0 + GPC) * HD])
        al = np.empty((128, CC * GPC), dtype=np.float32)
        for cc in range(CC):
            for gi in range(GPC):
                al[:, cc * GPC + gi] = w_head[g0 + gi, cc * 2 + p // 64]
        in_maps.append({
            "x": xc, "wqT": wqT, "wkT": wkT, "wvT": wvT,
            "wpT": wpT_r, "alphas": al,
            "ones": np.ones((128, 128), dtype=np.float32),
        })
    return in_maps


def _host_finish(results, x, w_v, w_head, in_gamma, in_beta, w_proj, b_proj):
    in_gamma = np.asarray(in_gamma, dtype=np.float32)
    in_beta = np.asarray(in_beta, dtype=np.float32)
    w_proj = np.asarray(w_proj, dtype=np.float32)
    b_proj = np.asarray(b_proj, dtype=np.float32)
    w_v = np.asarray(w_v, dtype=np.float32)
    x = np.asarray(x, dtype=np.float32)

    # collapsed_wp[d, c] = sum_jh w_proj[c, jh*64+d]
    collapsed_wp = w_proj.reshape(C, 8, 64).sum(axis=1).T   # [64, C]
    out = np.empty((B, C, T), dtype=np.float32)
    for core in range(NCORES):
        b = core // 2
        g0 = (core % 2) * GPC
        dev = results[core]["out"]              # [512 i, 512 c]
        s2 = results[core]["s2"].sum(axis=1)    # [GPC]
        sv = w_v @ x[b].reshape(C, T).sum(axis=1)   # [C]
        for gi in range(GPC):
            g = g0 + gi
            var = s2[gi] / float(T * T) - MU * MU
            a = in_gamma[g] / np.sqrt(var + EPS)
            cs = in_beta[g] - a * MU
            bias2 = collapsed_wp.T @ sv[g * HD:(g + 1) * HD]   # [C]
            blk = dev[gi * 128:(gi + 1) * 128, :]              # [128 i, C]
            full = a * blk + (cs * bias2 + b_proj)[None, :]
            out[b, :, g * 128:(g + 1) * 128] = full.T
    return out.reshape(B, C, HH, WW)


def _run(inputs, trace=False, reps=1):
    nc = build_program(reps)
    in_maps = _host_prep(**inputs)
    res = run_bass_kernel_spmd(nc, in_maps, core_ids=list(range(NCORES)),
                               trace=trace)
    out = _host_finish(res.results, inputs["x"], inputs["w_v"],
                       inputs["w_head"], inputs["in_gamma"],
                       inputs["in_beta"], inputs["w_proj"], inputs["b_proj"])
    return out, res


def kernel(**inputs):
    out, _ = _run(inputs, trace=False)
    return out

